# revision 12
# baseline (speedup 1.0000x reference)
"""Complex LayerNorm TRN2 kernel (nn_Complex_LayerNorm).

Math (per row r over embed dim D, per feature d):
    whiten:  y = C(r) @ (x - mu(r)),  C = inv(sqrtm(cov2x2))
    recolor: z = Wsqrt(d) @ y + bias(d)

Per-core design (pure data-parallel over batch, 1 batch row block per core):

  - Inputs are cast-loaded f32->f16 by the Pool-engine (SWDGE) DMA, halving
    the load-side traffic; all matmul operands are fp16 (~5e-4 rounding vs
    the 2e-2 rel-err budget), PSUM accumulation stays fp32.
  - Moments per chunk of CH_IN row tiles on DVE (bn_stats for x_real and
    x_imag, fused cross product with accumulate), then the 2x2 inverse-sqrt
    whitening coefficients are computed with tensor_tensor ops batched
    across the chunk (CH_IN values per op).
  - stage1:  psum1 = (xr - mu_r)_blk^T @ [diag(i00)|diag(i01)]
           + (xi - mu_i)_blk^T @ [diag(i01)|diag(i11)]
      (mean subtracted elementwise on DVE at 2x fp16 rate beforehand)
  - stage2:  psum2 = yrT_blk @ W1[b] + yiT_blk @ W2[b] + ones x brbi
      where W1/W2 are interleaved double-diagonal matrices built on-chip;
      they transpose back to row-major AND recolor AND interleave (zr,zi)
      pairs; the rank-1 term adds the bias.
  - PSUM->SBUF moves run on the Activation engine; the fp16 (zr,zi) pairs
    are DMA'd out by SWDGE and converted to complex64 on the host.
"""

import numpy as np

import concourse.bacc as bacc
import concourse.tile as tile
from concourse import mybir
from concourse import bass_utils

F32 = mybir.dt.float32
F32R = mybir.dt.float32r
F16 = mybir.dt.float16
AL = mybir.AluOpType
AF = mybir.ActivationFunctionType

B, S, D = 8, 4096, 1024
R = S                 # rows per core (batch dim sharded 1 per core)
NT = R // 128         # 32 row tiles
NB = D // 128         # 8 feature blocks
CH_IN = 8             # row tiles per input DMA / stats chunk
CH_OUT = 4            # row tiles per output DMA
C1 = 1024.0 / 1023.0  # unbiased variance correction (torch.var ddof=1)


def _build_nc(nt=NT):
    nc = bacc.Bacc("TRN2")

    xr_d = nc.dram_tensor("x_real", (nt * 128, D), F32, kind="ExternalInput").ap()
    xi_d = nc.dram_tensor("x_imag", (nt * 128, D), F32, kind="ExternalInput").ap()
    wv_d = nc.dram_tensor("wvecs", (128, 3, NB), F32, kind="ExternalInput").ap()
    bb_d = nc.dram_tensor("brbi", (1, 2 * D), F32R, kind="ExternalInput").ap()
    id_d = nc.dram_tensor("ident", (128, 128), F32, kind="ExternalInput").ap()
    ones_d = nc.dram_tensor("onesr", (1, 128), F32R, kind="ExternalInput").ap()
    out_d = nc.dram_tensor("out", (nt * 128, 2 * D), F16, kind="ExternalOutput").ap()

    with tile.TileContext(nc) as tc:
        with (
            tc.tile_pool(name="const", bufs=1) as pc,
            tc.tile_pool(name="xin", bufs=3) as px,
            tc.tile_pool(name="xmu", bufs=3) as pxm,
            tc.tile_pool(name="scratch", bufs=2) as psc,
            tc.tile_pool(name="stats", bufs=3) as pst,
            tc.tile_pool(name="diag", bufs=3) as pdg,
            tc.tile_pool(name="yt", bufs=4) as pyt,
            tc.tile_pool(name="outp", bufs=3) as pout,
            tc.tile_pool(name="ps1", bufs=2, space="PSUM") as ps1,
            tc.tile_pool(name="ps2", bufs=2, space="PSUM") as ps2,
        ):
            # ---- small constants via DMA ----
            wv = pc.tile([128, 3, NB], F32)
            nc.sync.dma_start(out=wv, in_=wv_d)
            brbi = pc.tile([1, 2 * D], F32R)
            nc.sync.dma_start(out=brbi, in_=bb_d)
            ident = pc.tile([128, 128], F32)
            nc.sync.dma_start(out=ident, in_=id_d)
            onesr = pc.tile([1, 128], F32R)
            nc.sync.dma_start(out=onesr, in_=ones_d)

            # ---- prefetch the first input chunks before the on-chip const
            # builds so the Pool engine's SWDGE cast-loads start immediately ----
            NCH = nt // CH_IN
            lead = {}

            def issue_loads(g):
                r0 = g * CH_IN * 128
                xrch = px.tile([128, CH_IN, D], F16, tag="xr")
                nc.gpsimd.dma_start(
                    out=xrch,
                    in_=xr_d[r0 : r0 + CH_IN * 128, :].rearrange(
                        "(c p) d -> p c d", p=128))
                xich = px.tile([128, CH_IN, D], F16, tag="xi")
                nc.gpsimd.dma_start(
                    out=xich,
                    in_=xi_d[r0 : r0 + CH_IN * 128, :].rearrange(
                        "(c p) d -> p c d", p=128))
                lead[g] = (xrch, xich)

            issue_loads(0)
            if NCH > 1:
                issue_loads(1)

            # ---- on-chip builds (once): W1/W2 interleaved double-diagonals,
            # [p, b, j, slot]: W1[p,b,j,0] = w00[128b+p]*ident[p,j], slot 1 =
            # w01; W2 slots = (w01, w11) ----
            w1c = pc.tile([128, NB, 128, 2], F32R)
            w2c = pc.tile([128, NB, 128, 2], F32R)
            for gb in range(NB):
                nc.gpsimd.tensor_scalar(
                    out=w1c[:, gb, :, 0], in0=ident,
                    scalar1=wv[:, 0, gb : gb + 1], scalar2=None, op0=AL.mult)
                nc.gpsimd.tensor_scalar(
                    out=w1c[:, gb, :, 1], in0=ident,
                    scalar1=wv[:, 1, gb : gb + 1], scalar2=None, op0=AL.mult)
                nc.gpsimd.tensor_scalar(
                    out=w2c[:, gb, :, 0], in0=ident,
                    scalar1=wv[:, 1, gb : gb + 1], scalar2=None, op0=AL.mult)
                nc.gpsimd.tensor_scalar(
                    out=w2c[:, gb, :, 1], in0=ident,
                    scalar1=wv[:, 2, gb : gb + 1], scalar2=None, op0=AL.mult)

            out_sb = None
            for g in range(NCH):
                if g + 2 < NCH:
                    issue_loads(g + 2)
                xrch, xich = lead.pop(g)

                # ---- moments for the whole chunk (DVE) ----
                ST = pst.tile([128, CH_IN, 5], F32, tag="st")
                for ci in range(CH_IN):
                    xr = xrch[:, ci, :]
                    xi = xich[:, ci, :]
                    bs = pst.tile([128, 2, 6], F32, tag="bsr")
                    nc.vector.bn_stats(out=bs[:, 0, :], in_=xr[:, 0:512])
                    nc.vector.bn_stats(out=bs[:, 1, :], in_=xr[:, 512:1024])
                    nc.vector.bn_aggr(out=ST[:, ci, 0:2], in_=bs)  # mu_r, var_r
                    bs2 = pst.tile([128, 2, 6], F32, tag="bsi")
                    nc.vector.bn_stats(out=bs2[:, 0, :], in_=xi[:, 0:512])
                    nc.vector.bn_stats(out=bs2[:, 1, :], in_=xi[:, 512:1024])
                    nc.vector.bn_aggr(out=ST[:, ci, 2:4], in_=bs2)  # mu_i, var_i
                    prod = psc.tile([128, D], F16, tag="prod")
                    nc.vector.scalar_tensor_tensor(
                        out=prod, in0=xr, scalar=1.0, in1=xi,
                        op0=AL.mult, op1=AL.mult,
                        accum_out=ST[:, ci, 4:5])  # sum(xr*xi)

                # ---- whitening coefficients, batched across the chunk ----
                tt = nc.vector.tensor_tensor
                tsv = nc.vector.tensor_scalar
                CC = pst.tile([128, 14, CH_IN], F32, tag="cc")
                # rows: 0 sri/D, 1 m, 2 cov, 3 p, 4 det, 5 s, 6 2s, 7 tsq,
                #       8 t*s, 9 inv, 10 a1, 11 i00, 12 i01(unnegated), 13 i11
                tsv(out=CC[:, 0, :], in0=ST[:, :, 4], scalar1=1.0 / D, scalar2=None, op0=AL.mult)
                tt(out=CC[:, 1, :], in0=ST[:, :, 0], in1=ST[:, :, 2], op=AL.mult)
                tt(out=CC[:, 2, :], in0=CC[:, 0, :], in1=CC[:, 1, :], op=AL.subtract)   # cov
                tt(out=CC[:, 3, :], in0=ST[:, :, 1], in1=ST[:, :, 3], op=AL.mult)
                tsv(out=CC[:, 3, :], in0=CC[:, 3, :], scalar1=C1 * C1, scalar2=None, op0=AL.mult)
                tt(out=CC[:, 4, :], in0=CC[:, 2, :], in1=CC[:, 2, :], op=AL.mult)
                tt(out=CC[:, 4, :], in0=CC[:, 3, :], in1=CC[:, 4, :], op=AL.subtract)   # det
                nc.scalar.activation(out=CC[:, 5, :], in_=CC[:, 4, :], func=AF.Sqrt)    # s
                tsv(out=CC[:, 6, :], in0=CC[:, 5, :], scalar1=2.0, scalar2=None, op0=AL.mult)
                tt(out=CC[:, 7, :], in0=ST[:, :, 1], in1=ST[:, :, 3], op=AL.add)
                tsv(out=CC[:, 7, :], in0=CC[:, 7, :], scalar1=C1, scalar2=None, op0=AL.mult)
                tt(out=CC[:, 7, :], in0=CC[:, 7, :], in1=CC[:, 6, :], op=AL.add)        # tsq
                nc.scalar.activation(out=CC[:, 8, :], in_=CC[:, 7, :], func=AF.Sqrt)    # t
                tt(out=CC[:, 8, :], in0=CC[:, 8, :], in1=CC[:, 5, :], op=AL.mult)       # t*s
                nc.vector.reciprocal(out=CC[:, 9, :], in_=CC[:, 8, :])                  # inv
                tsv(out=CC[:, 10, :], in0=ST[:, :, 3], scalar1=C1, scalar2=None, op0=AL.mult)
                tt(out=CC[:, 10, :], in0=CC[:, 10, :], in1=CC[:, 5, :], op=AL.add)
                tt(out=CC[:, 11, :], in0=CC[:, 10, :], in1=CC[:, 9, :], op=AL.mult)     # i00
                tt(out=CC[:, 12, :], in0=CC[:, 2, :], in1=CC[:, 9, :], op=AL.mult)      # i01
                tsv(out=CC[:, 13, :], in0=ST[:, :, 1], scalar1=C1, scalar2=None, op0=AL.mult)
                tt(out=CC[:, 13, :], in0=CC[:, 13, :], in1=CC[:, 5, :], op=AL.add)
                tt(out=CC[:, 13, :], in0=CC[:, 13, :], in1=CC[:, 9, :], op=AL.mult)     # i11

                for ci in range(CH_IN):
                    it = g * CH_IN + ci
                    xr = xrch[:, ci, :]
                    xi = xich[:, ci, :]

                    # ---- per-row diagonal matrices (Pool), fp16 ----
                    DG = pdg.tile([128, 3, 128], F16, tag="dg")
                    nc.gpsimd.tensor_scalar(out=DG[:, 0, :], in0=ident, scalar1=CC[:, 11, ci : ci + 1], scalar2=None, op0=AL.mult)
                    nc.gpsimd.tensor_scalar(out=DG[:, 1, :], in0=ident, scalar1=CC[:, 12, ci : ci + 1], scalar2=-1.0, op0=AL.mult, op1=AL.mult)
                    nc.gpsimd.tensor_scalar(out=DG[:, 2, :], in0=ident, scalar1=CC[:, 13, ci : ci + 1], scalar2=None, op0=AL.mult)

                    # ---- mean subtract (DVE, fp16 at 2x rate) ----
                    xmu = pxm.tile([128, 2, D], F16, tag="xm")
                    nc.vector.tensor_scalar(
                        out=xmu[:, 0, :], in0=xr,
                        scalar1=ST[:, ci, 0:1], scalar2=None, op0=AL.subtract)
                    nc.vector.tensor_scalar(
                        out=xmu[:, 1, :], in0=xi,
                        scalar1=ST[:, ci, 2:3], scalar2=None, op0=AL.subtract)

                    co = it % CH_OUT
                    if co == 0:
                        out_sb = pout.tile([128, CH_OUT, 2 * D], F16, tag="osb")

                    for h in range(2):  # halftiles (512 feats each)
                        p1 = ps1.tile([128, 1024], F32, tag="p1")
                        for b in range(4):
                            gb = 4 * h + b
                            o = p1[:, 256 * b : 256 * (b + 1)]
                            nc.tensor.matmul(o, xmu[:, 0, 128 * gb : 128 * (gb + 1)], DG[:, 0:2, :], start=True, stop=False)
                            nc.tensor.matmul(o, xmu[:, 1, 128 * gb : 128 * (gb + 1)], DG[:, 1:3, :], start=False, stop=True)

                        yt = pyt.tile([128, 1024], F32R, tag="yt")
                        nc.scalar.copy(out=yt, in_=p1)

                        p2 = ps2.tile([128, 1024], F32, tag="p2")
                        for k in range(2):  # psum banks
                            for j in range(2):
                                b = 2 * k + j
                                gb = 4 * h + b
                                o = p2[:, 256 * b : 256 * (b + 1)]
                                yrT = yt[:, 256 * b : 256 * b + 128]
                                yiT = yt[:, 256 * b + 128 : 256 * b + 256]
                                nc.tensor.matmul(o, yrT, w1c[:, gb, :, :], start=(j == 0), stop=False)
                                nc.tensor.matmul(o, yiT, w2c[:, gb, :, :], start=False, stop=False)
                            c0 = 1024 * h + 512 * k
                            nc.tensor.matmul(
                                p2[:, 512 * k : 512 * (k + 1)],
                                onesr, brbi[:, c0 : c0 + 512],
                                start=False, stop=True)

                        # fp16 downconvert in the PSUM->SBUF copy (Act)
                        nc.scalar.copy(
                            out=out_sb[:, co, 1024 * h : 1024 * (h + 1)], in_=p2)

                    if co == CH_OUT - 1:
                        g0 = (it - co) * 128
                        nc.gpsimd.dma_start(
                            out=out_d[g0 : g0 + CH_OUT * 128, :].rearrange(
                                "(c p) d -> p c d", p=128),
                            in_=out_sb)

    nc.finalize()
    return nc


_NC = None


def _get_nc():
    global _NC
    if _NC is None:
        _NC = _build_nc()
    return _NC


def _host_consts(weights, bias_real, bias_imag):
    w = weights.astype(np.float64)
    wr = w[:, 0, 0] ** 2
    wi = w[:, 1, 0] ** 2
    sig = 1.0 / (1.0 + np.exp(-w[:, 2, 0]))
    wc = (sig - 0.5) * 2.0 * np.sqrt(wr * wi)
    sw = np.sqrt(wr * wi - wc * wc)
    tw = np.sqrt(wr + wi + 2.0 * sw)
    w00 = ((wr + sw) / tw).astype(np.float32)
    w01 = (wc / tw).astype(np.float32)
    w11 = ((wi + sw) / tw).astype(np.float32)

    # wvecs[p, k, b] = w{k}[128*b + p] for k in (00, 01, 11)
    WV = np.empty((128, 3, NB), np.float32)
    WV[:, 0, :] = w00.reshape(NB, 128).T
    WV[:, 1, :] = w01.reshape(NB, 128).T
    WV[:, 2, :] = w11.reshape(NB, 128).T

    BRBI = np.empty((1, 2 * D), np.float32)
    BRBI[0, 0::2] = bias_real
    BRBI[0, 1::2] = bias_imag

    consts = {
        "wvecs": WV,
        "brbi": BRBI,
        "ident": np.eye(128, dtype=np.float32),
        "onesr": np.ones((1, 128), np.float32),
    }
    return consts


def _run(x_real, x_imag, weights, bias_real, bias_imag, trace=False):
    nc = _get_nc()
    consts = _host_consts(
        np.asarray(weights, np.float32),
        np.asarray(bias_real, np.float32),
        np.asarray(bias_imag, np.float32),
    )
    xr = np.ascontiguousarray(np.asarray(x_real, np.float32))
    xi = np.ascontiguousarray(np.asarray(x_imag, np.float32))
    in_maps = [
        {"x_real": xr[c], "x_imag": xi[c], **consts} for c in range(B)
    ]
    res = bass_utils.run_bass_kernel_spmd(
        nc, in_maps, core_ids=list(range(B)), trace=trace
    )
    out = np.empty((B, S, D), np.complex64)
    for c in range(B):
        pairs = np.ascontiguousarray(res.results[c]["out"]).astype(np.float32)
        out[c] = pairs.view(np.complex64)
    return out, res


def kernel(x_real, x_imag, weights, bias_real, bias_imag, trace=False):
    out, _ = _run(x_real, x_imag, weights, bias_real, bias_imag, trace=False)
    return out
